# revision 1
# baseline (speedup 1.0000x reference)
"""MMD (Maximum Mean Discrepancy) loss kernel for Trainium2, 8 NeuronCores.

Math: with x = concat(source, target) [N=8192, D=256],
  L2_ij = sq_i + sq_j - 2 x_i.x_j
  bandwidth = sum(L2) / (N^2-N) / 4   (closed form: sum(L2) = 2N*sum(sq) - 2||colsum x||^2)
  K = sum_b exp(-L2 / (bandwidth * 2^b)), b = 0..4
  loss = mean(K_SS) + mean(K_TT) - 2 mean(K_ST)

Strategy (triangle sharding over 512x512 tiles; K is symmetric so only the
upper triangle of the 16x16 tile grid is computed — 136 tiles instead of 256):
  total = sum_SS + sum_TT - 2 sum_ST.  Core k owns 17 tiles: SS row-block k
  (diag w=+1, 7-k uppers w=+2), TT row-block 7-k (diag w=+1, k uppers w=+2),
  ST row-block k (8 tiles, w=-2).  Same instruction count per core (SPMD);
  all per-core structure lives in host-packed tensors.

Per tile t and 128-row sub-block ib (68 units/core):
  PE:  G_ij = x_i.x_j - 0.5 sq_i - 0.5 sq_j  (= -L2/2), float32r (full rate),
       via K = 128 + 128 + 2 matmuls; the K=2 matmul contracts two augmented
       rows packed on SBUF partitions 0-1 (ones x v_j + u_i x ones).
  ACT: exp(2 a_b * G) = exp(-a_b L2) with per-partition scale AP; the fused
       accum_out reduces each [128, 2048] PSUM group (4 sub-blocks of t) to
       per-partition sums.  The 5 bandwidths are a geometric ladder
       (a_{b+1} = a_b/2), so with w = exp(-a_4 L2) the b=3 term is w^2:
       ScalarE computes only 4 exps (b=0,1,2 and w); VectorE produces w^2
       and its row-sum in one fused affine_mul_reduce.  ScalarE remains the
       bottleneck at 4 passes x N^2/2 elements x 1 elem/cycle/lane.
Host computes the bandwidth in closed form (fp64), packs per-core tiles,
applies tile weights, divides by B^2.

Notes:
  - ST tiles (t>=9) and the SS diag tile share slab-P rows, so their lhsT
    aliases rhs tile 0; only tiles 2..8 carry dedicated weights.
  - Weights/aug are float32r end-to-end (walrus requires producer dtype
    consistency for fp32r matmul inputs).
"""

import numpy as np

B = 4096
D = 256
N = 2 * B
KERNEL_MUL = 2.0
KERNEL_NUM = 5
NCORES = 8
TS = 512  # tile edge
NTILES = 17  # tiles per core
NIB = 4  # 128-row sub-blocks per tile
NWB = 7  # class-B tiles (t=2..8) with dedicated weights
NUSLOT = 8 + NWB * NIB  # distinct (slab, ib) u-row slots: A(8) + B(28)

_CACHE = {}


def _uslot(t, ib):
    """Unit -> slot in the deduplicated u-region of aug2."""
    if t < 2:
        return t * NIB + ib  # A: SSd -> P slots 0-3, TTd -> Q slots 4-7
    if t <= 8:
        return 8 + (t - 2) * NIB + ib  # B: per-tile slots
    return ib  # C (ST): slab P == slots 0-3


def _build_program(repeat=1):
    """Build the SPMD program. repeat>1 wraps the compute body in a hardware
    For loop (identical result; used only for differential HW timing)."""
    import concourse.bass as bass
    import concourse.tile as tile
    from concourse import bacc, mybir

    f32 = mybir.dt.float32
    f32r = mybir.dt.float32r
    bf16 = mybir.dt.bfloat16
    Exp = mybir.ActivationFunctionType.Exp

    nc = bacc.Bacc(None)

    xT = nc.declare_dram_parameter("xT", [128, NTILES, 2, TS], f32r, isOutput=False)
    wT = nc.declare_dram_parameter("wT", [128, NWB * NIB, 2, 128], f32r, isOutput=False)
    # aug2 row layout: cols [0, NUSLOT*128): (ones, u_i) per u-slot;
    # cols [NUSLOT*128, +NTILES*TS): (v_j, ones) per tile.
    AUGW = NUSLOT * 128 + NTILES * TS
    aug = nc.declare_dram_parameter("aug2", [2, AUGW], f32r, isOutput=False)
    scl = nc.declare_dram_parameter("scale", [128, KERNEL_NUM], f32, isOutput=False)
    res = nc.declare_dram_parameter("res", [128, NTILES * KERNEL_NUM], f32, isOutput=True)

    with tile.TileContext(nc) as tc:
        with (
            tc.tile_pool(name="sing", bufs=1) as sing,
            tc.tile_pool(name="scr", bufs=2) as scr,
            tc.tile_pool(name="psum", bufs=2, space=bass.MemorySpace.PSUM) as psum,
        ):
            rhs_sb = sing.tile([128, NTILES, 2, TS], f32r)
            w_sb = sing.tile([128, NWB * NIB, 2, 128], f32r)
            aug_sb = sing.tile([2, AUGW], f32r)
            scale_sb = sing.tile([128, KERNEL_NUM], f32)
            res_sb = sing.tile([128, NTILES * KERNEL_NUM], f32)

            nc.sync.dma_start(out=scale_sb, in_=scl[:])
            nc.sync.dma_start(out=aug_sb, in_=aug[:])
            for t in range(NTILES):
                nc.sync.dma_start(out=rhs_sb[:, t], in_=xT[:, t])
                if 2 <= t <= 8:
                    nc.sync.dma_start(
                        out=w_sb[:, (t - 2) * NIB : (t - 1) * NIB],
                        in_=wT[:, (t - 2) * NIB : (t - 1) * NIB],
                    )

            def body():
                for t in range(NTILES):
                    pt = psum.tile([128, NIB * TS], f32, tag="pt")
                    for ib in range(NIB):
                        sl = pt[:, ib * TS : (ib + 1) * TS]
                        if t < 2:
                            lhs0 = rhs_sb[:, t, 0, ib * 128 : (ib + 1) * 128]
                            lhs1 = rhs_sb[:, t, 1, ib * 128 : (ib + 1) * 128]
                        elif t <= 8:
                            lhs0 = w_sb[:, (t - 2) * NIB + ib, 0]
                            lhs1 = w_sb[:, (t - 2) * NIB + ib, 1]
                        else:  # ST: slab-P rows == tile-0 columns
                            lhs0 = rhs_sb[:, 0, 0, ib * 128 : (ib + 1) * 128]
                            lhs1 = rhs_sb[:, 0, 1, ib * 128 : (ib + 1) * 128]
                        us = _uslot(t, ib)
                        nc.tensor.matmul(sl, lhs0, rhs_sb[:, t, 0], start=True, stop=False)
                        nc.tensor.matmul(sl, lhs1, rhs_sb[:, t, 1], start=False, stop=False)
                        nc.tensor.matmul(
                            sl,
                            aug_sb[:, us * 128 : (us + 1) * 128],
                            aug_sb[:, NUSLOT * 128 + t * TS : NUSLOT * 128 + (t + 1) * TS],
                            start=False,
                            stop=True,
                        )
                    # b = 0, 1, 2: plain exp passes on ScalarE
                    for b in (0, 1, 2):
                        sc = scr.tile([128, NIB * TS], bf16, tag="sc")
                        nc.scalar.activation(
                            out=sc[:],
                            in_=pt[:],
                            func=Exp,
                            scale=scale_sb[:, b : b + 1],
                            accum_out=res_sb[
                                :, t * KERNEL_NUM + b : t * KERNEL_NUM + b + 1
                            ],
                        )
                    # b = 4: w = exp(-a4 L2) kept in fp32; b=3 (w^2) and
                    # b=2 (w^4 = (w^2)^2) come from fused DVE square+reduce.
                    w_t = scr.tile([128, NIB * TS], f32, tag="w")
                    nc.scalar.activation(
                        out=w_t[:],
                        in_=pt[:],
                        func=Exp,
                        scale=scale_sb[:, 4:5],
                        accum_out=res_sb[:, t * KERNEL_NUM + 4 : t * KERNEL_NUM + 5],
                    )
                    w2_t = scr.tile([128, NIB * TS], f32, tag="w2")
                    nc.vector.affine_mul_reduce(
                        out=w2_t[:],
                        accum_out=res_sb[:, t * KERNEL_NUM + 3 : t * KERNEL_NUM + 4],
                        in0=w_t[:],
                        in1=w_t[:],
                        scale=1.0,
                        bias=0.0,
                    )


            if repeat == 1:
                body()
            else:
                with tc.For_i(0, repeat) as _i:
                    body()

            nc.sync.dma_start(out=res[:], in_=res_sb[:])

    nc.finalize()
    return nc


def _get_program():
    if "nc" not in _CACHE:
        _CACHE["nc"] = _build_program()
    return _CACHE["nc"]


def _core_tiles(k):
    """Per-core tile list: (rowbase, colbase, weight). Order defines t."""
    P = TS * k  # S row-block k
    Q = B + TS * (7 - k)  # T row-block 7-k
    tiles = [(P, P, 1.0), (Q, Q, 1.0)]  # SSd, TTd
    for j in range(k + 1, 8):  # SS+ (7-k tiles)
        tiles.append((P, TS * j, 2.0))
    for j in range(8 - k, 8):  # TT+ (k tiles)
        tiles.append((Q, B + TS * j, 2.0))
    for j in range(8):  # ST (8 tiles)
        tiles.append((P, B + TS * j, -2.0))
    assert len(tiles) == NTILES
    return tiles


def _host_prep(source_features, target_features):
    x = np.concatenate(
        [np.asarray(source_features, np.float32), np.asarray(target_features, np.float32)],
        axis=0,
    )  # [N, D]
    x64 = x.astype(np.float64)
    sq = np.sum(x64 * x64, axis=1)
    colsum = np.sum(x64, axis=0)
    sum_l2 = 2.0 * N * np.sum(sq) - 2.0 * np.dot(colsum, colsum)
    bandwidth = sum_l2 / (N * N - N) / (KERNEL_MUL ** (KERNEL_NUM // 2))
    a = np.array([1.0 / (bandwidth * KERNEL_MUL**b) for b in range(KERNEL_NUM)])

    xt = np.ascontiguousarray(x.T)  # [D, N]
    sqf = sq.astype(np.float32)
    scale_host = np.broadcast_to((2.0 * a).astype(np.float32), (128, KERNEL_NUM)).copy()
    AUGW = NUSLOT * 128 + NTILES * TS

    in_maps = []
    for k in range(NCORES):
        tiles = _core_tiles(k)
        rhs_host = np.empty((128, NTILES, 2, TS), np.float32)
        w_host = np.empty((128, NWB * NIB, 2, 128), np.float32)
        aug_host = np.empty((2, AUGW), np.float32)
        for t, (rb, cb, _w) in enumerate(tiles):
            rhs_host[:, t, 0, :] = xt[0:128, cb : cb + TS]
            rhs_host[:, t, 1, :] = xt[128:256, cb : cb + TS]
            v0 = NUSLOT * 128 + t * TS
            aug_host[0, v0 : v0 + TS] = -0.5 * sqf[cb : cb + TS]
            aug_host[1, v0 : v0 + TS] = 1.0
            for ib in range(NIB):
                r0 = rb + ib * 128
                us = _uslot(t, ib)
                aug_host[0, us * 128 : (us + 1) * 128] = 1.0
                aug_host[1, us * 128 : (us + 1) * 128] = -0.5 * sqf[r0 : r0 + 128]
                if 2 <= t <= 8:
                    w_host[:, (t - 2) * NIB + ib, 0, :] = xt[0:128, r0 : r0 + 128]
                    w_host[:, (t - 2) * NIB + ib, 1, :] = xt[128:256, r0 : r0 + 128]
        in_maps.append(
            {"xT": rhs_host, "wT": w_host, "aug2": aug_host, "scale": scale_host}
        )
    return in_maps


def _combine(results):
    total = 0.0
    for k in range(NCORES):
        r = np.asarray(results[k]["res"], np.float64).reshape(128, NTILES, KERNEL_NUM)
        s_t = r.sum(axis=(0, 2))
        w = np.array([w for (_rb, _cb, w) in _core_tiles(k)])
        total += float(np.dot(w, s_t))
    return np.float32(total / (B * B))


def kernel(source_features, target_features):
    from concourse.bass_utils import run_bass_kernel_spmd

    nc = _get_program()
    in_maps = _host_prep(source_features, target_features)
    out = run_bass_kernel_spmd(nc, in_maps, list(range(NCORES)))
    return _combine(out.results)



# revision 4
# speedup vs baseline: 1.5895x; 1.5895x over previous
"""MMD (Maximum Mean Discrepancy) loss kernel for Trainium2, 8 NeuronCores.

Math: with x = concat(source, target) [N=8192, D=256],
  L2_ij = sq_i + sq_j - 2 x_i.x_j
  bandwidth = sum(L2) / (N^2-N) / 4   (closed form: sum(L2) = 2N*sum(sq) - 2||colsum x||^2)
  K = sum_b exp(-L2 / (bandwidth * 2^b)), b = 0..4
  loss = mean(K_SS) + mean(K_TT) - 2 mean(K_ST)

Strategy (triangle sharding over 512x512 tiles; K is symmetric so only the
upper triangle of the 16x16 tile grid is computed — 136 tiles instead of 256):
  total = sum_SS + sum_TT - 2 sum_ST.  Core k owns 17 tiles: SS row-block k
  (diag w=+1, 7-k uppers w=+2), TT row-block 7-k (diag w=+1, k uppers w=+2),
  ST row-block k (8 tiles, w=-2).  Same instruction count per core (SPMD);
  all per-core structure lives in host-packed tensors.

The 5 bandwidths are a geometric ladder (a_{b+1} = a_b/2), so with
e4 = exp(-a_4 L2) every other kernel is a square: e_b = e_{b+1}^2.  Only ONE
exp pass is needed; the rest are element squarings spread over three engines:
  PE:  G_ij = x_i.x_j - 0.5 sq_i - 0.5 sq_j  (= -L2/2), float32r (full rate),
       12 matmuls/tile (K = 128+128+2; the K=2 matmul contracts two augmented
       rows).  Plus 8-12 reduce-matmuls/tile: lhsT = w_t-weighted ones columns
       contract fp16 value tiles into a persistent [3,512] PSUM accumulator
       (rows: e3 / e1 / e0 sums), start=False across the whole body.
  ACT: e4 = exp(2 a4 G) from PSUM (2 x FD=1024, free accum -> b4 sums),
       e2 = Square(e3) (FD=2048 SBUF, free accum -> b2 sums).
  DVE: e3 = e4*e4, e1 = e2*e2 (or affine_mul_reduce), e0 = e1*e1 — plain
       fp16 tensor_tensor runs in 2x_1p mode (2 elem/cycle/lane); the fused
       reduce variants only have 1x uops, so sums ride the PE/ACT instead.
Value tiles are fp16 (not bf16): the chained squarings feed later sums, and
the loss is a small difference of large block means, so rounding noise
matters; fp16 keeps it ~4x lower.
Engine load/tile ~ ACT 4.2us, DVE 4.4us, PE 4.3-5.2us (vs 8.0us ACT-bound
baseline).  Emission is software-pipelined (reduces lag 1-2 tiles) so no
engine head-of-line blocks.
Host computes the bandwidth in closed form (fp64), packs per-core tiles,
applies tile weights, divides by B^2.
"""

import numpy as np

B = 4096
D = 256
N = 2 * B
KERNEL_MUL = 2.0
KERNEL_NUM = 5
NCORES = 8
TS = 512  # tile edge
NTILES = 17  # tiles per core
NIB = 4  # 128-row sub-blocks per tile
NWB = 7  # class-B tiles (t=2..8) with dedicated weights
NUSLOT = 8 + NWB * NIB  # distinct (slab, ib) u-row slots: A(8) + B(28)
NRES = 4  # res slots per tile: b4 (x2 halves), b2, b1
E1_VIA_PE = True  # False: e1 sum via affine_mul_reduce (DVE 1x) instead

_CACHE = {}


def _uslot(t, ib):
    """Unit -> slot in the deduplicated u-region of aug2."""
    if t < 2:
        return t * NIB + ib  # A: SSd -> P slots 0-3, TTd -> Q slots 4-7
    if t <= 8:
        return 8 + (t - 2) * NIB + ib  # B: per-tile slots
    return ib  # C (ST): slab P == slots 0-3


def _wclass(t):
    """Tile weight class: 0 -> +1 (diag), 1 -> +2 (upper), 2 -> -2 (ST)."""
    return 0 if t < 2 else (1 if t <= 8 else 2)


def _build_program(repeat=1):
    """Build the SPMD program. repeat>1 wraps the compute body in a hardware
    For loop (identical result; used only for differential HW timing)."""
    import concourse.bass as bass
    import concourse.tile as tile
    from concourse import bacc, mybir

    f32 = mybir.dt.float32
    f32r = mybir.dt.float32r
    f16 = mybir.dt.float16
    Exp = mybir.ActivationFunctionType.Exp
    Square = mybir.ActivationFunctionType.Square

    nc = bacc.Bacc(None)

    xT = nc.declare_dram_parameter("xT", [128, NTILES, 2, TS], f32r, isOutput=False)
    wT = nc.declare_dram_parameter("wT", [128, NWB * NIB, 2, 128], f32r, isOutput=False)
    # aug2 row layout: cols [0, NUSLOT*128): (ones, u_i) per u-slot;
    # cols [NUSLOT*128, +NTILES*TS): (v_j, ones) per tile.
    AUGW = NUSLOT * 128 + NTILES * TS
    aug = nc.declare_dram_parameter("aug2", [2, AUGW], f32r, isOutput=False)
    scl = nc.declare_dram_parameter("scale", [128, 1], f32, isOutput=False)
    # reduce weights: 9 variants (3 classes x 3 slots) of [128, 3] fp16 ones-columns
    rw = nc.declare_dram_parameter("rw", [128, 27], f16, isOutput=False)
    res = nc.declare_dram_parameter("res", [128, NTILES * NRES], f32, isOutput=True)
    accd = nc.declare_dram_parameter("accd", [3, TS], f32, isOutput=True)

    with tile.TileContext(nc) as tc:
        with (
            tc.tile_pool(name="sing", bufs=1) as sing,
            tc.tile_pool(name="scr", bufs=2) as scr,
            tc.tile_pool(name="psum", bufs=3, space=bass.MemorySpace.PSUM) as psum,
            tc.tile_pool(name="pacc", bufs=1, space=bass.MemorySpace.PSUM) as pacc,
        ):
            rhs_sb = sing.tile([128, NTILES, 2, TS], f32r)
            w_sb = sing.tile([128, NWB * NIB, 2, 128], f32r)
            aug_sb = sing.tile([2, AUGW], f32r)
            scale_sb = sing.tile([128, 1], f32)
            rw_sb = sing.tile([128, 27], f16)
            res_sb = sing.tile([128, NTILES * NRES], f32)
            acc_sb = sing.tile([3, TS], f32)
            acc_ps = pacc.tile([3, TS], f32)

            nc.sync.dma_start(out=scale_sb, in_=scl[:])
            nc.sync.dma_start(out=rw_sb, in_=rw[:])
            nc.sync.dma_start(out=aug_sb, in_=aug[:])
            for t in range(NTILES):
                nc.sync.dma_start(out=rhs_sb[:, t], in_=xT[:, t])
                if 2 <= t <= 8:
                    nc.sync.dma_start(
                        out=w_sb[:, (t - 2) * NIB : (t - 1) * NIB],
                        in_=wT[:, (t - 2) * NIB : (t - 1) * NIB],
                    )

            def body():
                pth = {}
                e4v, e3v, e2v, e1v, e0v = {}, {}, {}, {}, {}
                state = {"first": True}

                def emit_mains(t):
                    for h in (0, 1):
                        pt = psum.tile([128, 2 * TS], f32, tag="pt")
                        pth[(t, h)] = pt
                        for i2 in (0, 1):
                            ib = 2 * h + i2
                            sl = pt[:, i2 * TS : (i2 + 1) * TS]
                            if t < 2:
                                lhs0 = rhs_sb[:, t, 0, ib * 128 : (ib + 1) * 128]
                                lhs1 = rhs_sb[:, t, 1, ib * 128 : (ib + 1) * 128]
                            elif t <= 8:
                                lhs0 = w_sb[:, (t - 2) * NIB + ib, 0]
                                lhs1 = w_sb[:, (t - 2) * NIB + ib, 1]
                            else:  # ST: slab-P rows == tile-0 columns
                                lhs0 = rhs_sb[:, 0, 0, ib * 128 : (ib + 1) * 128]
                                lhs1 = rhs_sb[:, 0, 1, ib * 128 : (ib + 1) * 128]
                            us = _uslot(t, ib)
                            nc.tensor.matmul(sl, lhs0, rhs_sb[:, t, 0], start=True, stop=False)
                            nc.tensor.matmul(sl, lhs1, rhs_sb[:, t, 1], start=False, stop=False)
                            nc.tensor.matmul(
                                sl,
                                aug_sb[:, us * 128 : (us + 1) * 128],
                                aug_sb[:, NUSLOT * 128 + t * TS : NUSLOT * 128 + (t + 1) * TS],
                                start=False,
                                stop=True,
                            )

                def emit_exp(t):
                    ev = scr.tile([128, NIB * TS], f16, tag="e4")
                    e4v[t] = ev
                    for h in (0, 1):
                        nc.scalar.activation(
                            out=ev[:, h * 2 * TS : (h + 1) * 2 * TS],
                            in_=pth.pop((t, h))[:],
                            func=Exp,
                            scale=scale_sb[:, 0:1],
                            accum_out=res_sb[:, t * NRES + h : t * NRES + h + 1],
                        )

                def emit_e3(t):
                    e3 = scr.tile([128, NIB * TS], f16, tag="e3")
                    e3v[t] = e3
                    nc.vector.tensor_mul(e3[:], e4v[t][:], e4v[t][:])

                def emit_sq2(t):
                    e2 = scr.tile([128, NIB * TS], f16, tag="e2")
                    e2v[t] = e2
                    nc.scalar.activation(
                        out=e2[:],
                        in_=e3v[t][:],
                        func=Square,
                        accum_out=res_sb[:, t * NRES + 2 : t * NRES + 3],
                    )

                def emit_e1(t):
                    e1 = scr.tile([128, NIB * TS], f16, tag="e1")
                    e1v[t] = e1
                    if E1_VIA_PE:
                        nc.vector.tensor_mul(e1[:], e2v[t][:], e2v[t][:])
                    else:
                        nc.vector.affine_mul_reduce(
                            out=e1[:],
                            accum_out=res_sb[:, t * NRES + 3 : t * NRES + 4],
                            in0=e2v[t][:],
                            in1=e2v[t][:],
                            scale=1.0,
                            bias=0.0,
                        )

                def emit_e0(t):
                    e0 = scr.tile([128, NIB * TS], f16, tag="e0")
                    e0v[t] = e0
                    nc.vector.tensor_mul(e0[:], e1v[t][:], e1v[t][:])

                def emit_red(t, val, slot):
                    v = _wclass(t) * 3 + slot
                    lhs = rw_sb[:, v * 3 : v * 3 + 3]
                    last = slot == 2 and t == NTILES - 1
                    for k in range(NIB):
                        nc.tensor.matmul(
                            acc_ps[:],
                            lhs,
                            val[:, k * TS : (k + 1) * TS],
                            start=state["first"],
                            stop=last and k == NIB - 1,
                            skip_group_check=True,
                        )
                        state["first"] = False

                for u in range(NTILES + 2):
                    if u < NTILES:
                        emit_mains(u)
                        emit_exp(u)
                        emit_e3(u)
                    if 1 <= u <= NTILES:
                        t = u - 1
                        emit_sq2(t)
                        emit_e1(t)
                        emit_e0(t)
                        emit_red(t, e3v.pop(t), 0)
                    if u >= 2:
                        t = u - 2
                        if E1_VIA_PE:
                            emit_red(t, e1v.pop(t), 1)
                        emit_red(t, e0v.pop(t), 2)

            if repeat == 1:
                body()
            else:
                with tc.For_i(0, repeat) as _i:
                    body()

            nc.vector.tensor_copy(acc_sb[:], acc_ps[:])
            nc.sync.dma_start(out=res[:], in_=res_sb[:])
            nc.sync.dma_start(out=accd[:], in_=acc_sb[:])

    nc.finalize()
    return nc


def _get_program():
    if "nc" not in _CACHE:
        _CACHE["nc"] = _build_program()
    return _CACHE["nc"]


def _core_tiles(k):
    """Per-core tile list: (rowbase, colbase, weight). Order defines t."""
    P = TS * k  # S row-block k
    Q = B + TS * (7 - k)  # T row-block 7-k
    tiles = [(P, P, 1.0), (Q, Q, 1.0)]  # SSd, TTd
    for j in range(k + 1, 8):  # SS+ (7-k tiles)
        tiles.append((P, TS * j, 2.0))
    for j in range(8 - k, 8):  # TT+ (k tiles)
        tiles.append((Q, B + TS * j, 2.0))
    for j in range(8):  # ST (8 tiles)
        tiles.append((P, B + TS * j, -2.0))
    assert len(tiles) == NTILES
    return tiles


def _host_prep(source_features, target_features):
    x = np.concatenate(
        [np.asarray(source_features, np.float32), np.asarray(target_features, np.float32)],
        axis=0,
    )  # [N, D]
    x64 = x.astype(np.float64)
    sq = np.sum(x64 * x64, axis=1)
    colsum = np.sum(x64, axis=0)
    sum_l2 = 2.0 * N * np.sum(sq) - 2.0 * np.dot(colsum, colsum)
    bandwidth = sum_l2 / (N * N - N) / (KERNEL_MUL ** (KERNEL_NUM // 2))
    a4 = 1.0 / (bandwidth * KERNEL_MUL**4)

    xt = np.ascontiguousarray(x.T)  # [D, N]
    sqf = sq.astype(np.float32)
    scale_host = np.full((128, 1), 2.0 * a4, np.float32)
    rw_host = np.zeros((128, 27), np.float16)
    for c, w in enumerate((1.0, 2.0, -2.0)):
        for s in range(3):
            rw_host[:, (c * 3 + s) * 3 + s] = w
    AUGW = NUSLOT * 128 + NTILES * TS

    in_maps = []
    for k in range(NCORES):
        tiles = _core_tiles(k)
        rhs_host = np.empty((128, NTILES, 2, TS), np.float32)
        w_host = np.empty((128, NWB * NIB, 2, 128), np.float32)
        aug_host = np.empty((2, AUGW), np.float32)
        for t, (rb, cb, _w) in enumerate(tiles):
            rhs_host[:, t, 0, :] = xt[0:128, cb : cb + TS]
            rhs_host[:, t, 1, :] = xt[128:256, cb : cb + TS]
            v0 = NUSLOT * 128 + t * TS
            aug_host[0, v0 : v0 + TS] = -0.5 * sqf[cb : cb + TS]
            aug_host[1, v0 : v0 + TS] = 1.0
            for ib in range(NIB):
                r0 = rb + ib * 128
                us = _uslot(t, ib)
                aug_host[0, us * 128 : (us + 1) * 128] = 1.0
                aug_host[1, us * 128 : (us + 1) * 128] = -0.5 * sqf[r0 : r0 + 128]
                if 2 <= t <= 8:
                    w_host[:, (t - 2) * NIB + ib, 0, :] = xt[0:128, r0 : r0 + 128]
                    w_host[:, (t - 2) * NIB + ib, 1, :] = xt[128:256, r0 : r0 + 128]
        in_maps.append(
            {
                "xT": rhs_host,
                "wT": w_host,
                "aug2": aug_host,
                "scale": scale_host,
                "rw": rw_host,
            }
        )
    return in_maps


def _combine(results):
    total = 0.0
    for k in range(NCORES):
        r = np.asarray(results[k]["res"], np.float64).reshape(128, NTILES, NRES)
        # slots: 0,1 = b4 halves; 2 = b2; 3 = b1 (amr mode only)
        nslots = 3 if E1_VIA_PE else 4
        s_t = r[:, :, :nslots].sum(axis=(0, 2))
        w = np.array([w for (_rb, _cb, w) in _core_tiles(k)])
        total += float(np.dot(w, s_t))
        a = np.asarray(results[k]["accd"], np.float64)  # [3, TS] w-weighted
        total += float(a[0].sum() + a[2].sum())
        if E1_VIA_PE:
            total += float(a[1].sum())
    return np.float32(total / (B * B))


def kernel(source_features, target_features):
    from concourse.bass_utils import run_bass_kernel_spmd

    nc = _get_program()
    in_maps = _host_prep(source_features, target_features)
    out = run_bass_kernel_spmd(nc, in_maps, list(range(NCORES)))
    return _combine(out.results)


# revision 16
# speedup vs baseline: 1.8035x; 1.1346x over previous
"""MMD (Maximum Mean Discrepancy) loss kernel for Trainium2, 8 NeuronCores.

Math: with x = concat(source, target) [N=8192, D=256],
  L2_ij = sq_i + sq_j - 2 x_i.x_j
  bandwidth = sum(L2) / (N^2-N) / 4   (closed form: sum(L2) = 2N*sum(sq) - 2||colsum x||^2)
  K = sum_b exp(-L2 / (bandwidth * 2^b)), b = 0..4
  loss = mean(K_SS) + mean(K_TT) - 2 mean(K_ST)

Strategy (triangle sharding over 512x512 tiles; K is symmetric so only the
upper triangle of the 16x16 tile grid is computed — 136 tiles instead of 256):
  total = sum_SS + sum_TT - 2 sum_ST.  Core k owns 17 tiles: SS row-block k
  (diag w=+1, 7-k uppers w=+2), TT row-block 7-k (diag w=+1, k uppers w=+2),
  ST row-block k (8 tiles, w=-2).  Same instruction count per core (SPMD);
  all per-core structure lives in host-packed tensors.

The 5 bandwidths are a geometric ladder (a_{b+1} = a_b/2), so with
e4 = exp(-a_4 L2) every other kernel is a square: e_b = e_{b+1}^2.  Only ONE
exp pass is needed; the rest are element squarings spread over three engines:
  PE:  G_ij = x_i.x_j - 0.5 sq_i - 0.5 sq_j  (= -L2/2), float32r (full rate),
       12 matmuls/tile (K = 128+128+2; the K=2 matmul contracts two augmented
       rows).  Plus 8-12 reduce-matmuls/tile: lhsT = w_t-weighted ones columns
       contract fp16 value tiles into a persistent [3,512] PSUM accumulator
       (rows: e3 / e1 / e0 sums), start=False across the whole body.
  ACT: e4 = exp(2 a4 G) from PSUM (2 x FD=1024, free accum -> b4 sums),
       e2 = Square(e3) (FD=2048 SBUF, free accum -> b2 sums).
  DVE: e3 = e4*e4, e1 = e2*e2 (or affine_mul_reduce), e0 = e1*e1 — plain
       fp16 tensor_tensor runs in 2x_1p mode (2 elem/cycle/lane); the fused
       reduce variants only have 1x uops, so sums ride the PE/ACT instead.
Value tiles are fp16 (not bf16): the chained squarings feed later sums, and
the loss is a small difference of large block means, so rounding noise
matters; fp16 keeps it ~4x lower.
Engine load/tile ~ ACT 4.2us, DVE 4.4us, PE 4.3-5.2us (vs 8.0us ACT-bound
baseline).  Emission is software-pipelined (reduces lag 1-2 tiles) so no
engine head-of-line blocks.
Host computes the bandwidth in closed form (fp64), packs per-core tiles,
applies tile weights, divides by B^2.
"""

import numpy as np

B = 4096
D = 256
N = 2 * B
KERNEL_MUL = 2.0
KERNEL_NUM = 5
NCORES = 8
TS = 512  # tile edge
NTILES = 17  # tiles per core
NIB = 4  # 128-row sub-blocks per tile
NWB = 7  # class-B tiles (t=2..8) with dedicated weights
NUSLOT = 8 + NWB * NIB  # distinct (slab, ib) u-row slots: A(8) + B(28)
NRES = 4  # res slots per tile: b4 (x2 halves), b2, b1
E1_VIA_PE = True  # False: e1 sum via affine_mul_reduce (DVE 1x) instead

# Timing-probe knobs (wrong results when enabled; used only to attribute
# steady-state time to engines via differential runs).
PROBE = {"no_red": False, "no_vsq": False, "no_sq2": False}

_CACHE = {}


def _uslot(t, ib):
    """Unit -> slot in the deduplicated u-region of aug2."""
    if t < 2:
        return t * NIB + ib  # A: SSd -> P slots 0-3, TTd -> Q slots 4-7
    if t <= 8:
        return 8 + (t - 2) * NIB + ib  # B: per-tile slots
    return ib  # C (ST): slab P == slots 0-3


def _wclass(t):
    """Tile weight class: 0 -> +1 (diag), 1 -> +2 (upper), 2 -> -2 (ST)."""
    return 0 if t < 2 else (1 if t <= 8 else 2)


def _build_program(repeat=1):
    """Build the SPMD program. repeat>1 wraps the compute body in a hardware
    For loop (identical result; used only for differential HW timing)."""
    import concourse.bass as bass
    import concourse.tile as tile
    from concourse import bacc, mybir

    f32 = mybir.dt.float32
    f32r = mybir.dt.float32r
    f16 = mybir.dt.float16
    Exp = mybir.ActivationFunctionType.Exp
    Square = mybir.ActivationFunctionType.Square

    nc = bacc.Bacc(None)

    xT = nc.declare_dram_parameter("xT", [128, NTILES, 2, TS], f32r, isOutput=False)
    wT = nc.declare_dram_parameter("wT", [128, NWB * NIB, 2, 128], f32r, isOutput=False)
    # aug2 row layout: cols [0, NUSLOT*128): (ones, u_i) per u-slot;
    # cols [NUSLOT*128, +NTILES*TS): (v_j, ones) per tile.
    AUGW = NUSLOT * 128 + NTILES * TS
    aug = nc.declare_dram_parameter("aug2", [2, AUGW], f32r, isOutput=False)
    scl = nc.declare_dram_parameter("scale", [128, 1], f32, isOutput=False)
    # reduce weights: 3 classes of [128, 1] fp16 w*ones columns
    rw = nc.declare_dram_parameter("rw", [128, 3], f16, isOutput=False)
    res = nc.declare_dram_parameter("res", [128, NTILES * NRES], f32, isOutput=True)
    accd = nc.declare_dram_parameter("accd", [128, TS], f32, isOutput=True)

    with tile.TileContext(nc) as tc:
        with (
            tc.tile_pool(name="sing", bufs=1) as sing,
            tc.tile_pool(name="scr", bufs=2) as scr,
            tc.tile_pool(name="psum", bufs=3, space=bass.MemorySpace.PSUM) as psum,
            tc.tile_pool(name="pacc", bufs=1, space=bass.MemorySpace.PSUM) as pacc,
        ):
            rhs_sb = sing.tile([128, NTILES, 2, TS], f32r)
            w_sb = sing.tile([128, NWB * NIB, 2, 128], f32r)
            aug_sb = sing.tile([2, AUGW], f32r)
            scale_sb = sing.tile([128, 1], f32)
            rw_sb = sing.tile([128, 3], f16)
            res_sb = sing.tile([128, NTILES * NRES], f32)
            acc_sb = sing.tile([128, TS], f32)
            acc_ps = pacc.tile([128, TS], f32)

            nc.sync.dma_start(out=scale_sb, in_=scl[:])
            nc.sync.dma_start(out=rw_sb, in_=rw[:])
            nc.sync.dma_start(out=aug_sb, in_=aug[:])
            for t in range(NTILES):
                nc.sync.dma_start(out=rhs_sb[:, t], in_=xT[:, t])
                if 2 <= t <= 8:
                    nc.sync.dma_start(
                        out=w_sb[:, (t - 2) * NIB : (t - 1) * NIB],
                        in_=wT[:, (t - 2) * NIB : (t - 1) * NIB],
                    )

            def body():
                pth = {}
                e4v, e3v, e2v, e1v, e0v = {}, {}, {}, {}, {}
                state = {"first": [True] * NIB}

                def emit_mains(t):
                    for h in (0, 1):
                        pt = psum.tile([128, 2 * TS], f32, tag="pt")
                        pth[(t, h)] = pt
                        for i2 in (0, 1):
                            ib = 2 * h + i2
                            sl = pt[:, i2 * TS : (i2 + 1) * TS]
                            if t < 2:
                                lhs0 = rhs_sb[:, t, 0, ib * 128 : (ib + 1) * 128]
                                lhs1 = rhs_sb[:, t, 1, ib * 128 : (ib + 1) * 128]
                            elif t <= 8:
                                lhs0 = w_sb[:, (t - 2) * NIB + ib, 0]
                                lhs1 = w_sb[:, (t - 2) * NIB + ib, 1]
                            else:  # ST: slab-P rows == tile-0 columns
                                lhs0 = rhs_sb[:, 0, 0, ib * 128 : (ib + 1) * 128]
                                lhs1 = rhs_sb[:, 0, 1, ib * 128 : (ib + 1) * 128]
                            us = _uslot(t, ib)
                            nc.tensor.matmul(sl, lhs0, rhs_sb[:, t, 0], start=True, stop=False)
                            nc.tensor.matmul(sl, lhs1, rhs_sb[:, t, 1], start=False, stop=False)
                            nc.tensor.matmul(
                                sl,
                                aug_sb[:, us * 128 : (us + 1) * 128],
                                aug_sb[:, NUSLOT * 128 + t * TS : NUSLOT * 128 + (t + 1) * TS],
                                start=False,
                                stop=True,
                            )

                def emit_exp(t):
                    ev = scr.tile([128, NIB * TS], f16, tag="e4")
                    e4v[t] = ev
                    for h in (0, 1):
                        nc.scalar.activation(
                            out=ev[:, h * 2 * TS : (h + 1) * 2 * TS],
                            in_=pth.pop((t, h))[:],
                            func=Exp,
                            scale=scale_sb[:, 0:1],
                            accum_out=res_sb[:, t * NRES + h : t * NRES + h + 1],
                        )

                def emit_e3(t):
                    e3 = scr.tile([128, NIB * TS], f16, tag="e3")
                    e3v[t] = e3
                    if not PROBE["no_vsq"]:
                        nc.vector.tensor_mul(e3[:], e4v[t][:], e4v[t][:])

                def emit_sq2(t):
                    e2 = scr.tile([128, NIB * TS], f16, tag="e2")
                    e2v[t] = e2
                    if not PROBE["no_sq2"]:
                        nc.scalar.activation(
                            out=e2[:],
                            in_=e3v[t][:],
                            func=Square,
                            accum_out=res_sb[:, t * NRES + 2 : t * NRES + 3],
                        )

                def emit_e1(t):
                    e1 = scr.tile([128, NIB * TS], f16, tag="e1")
                    e1v[t] = e1
                    if PROBE["no_vsq"]:
                        return
                    if E1_VIA_PE:
                        nc.vector.tensor_mul(e1[:], e2v[t][:], e2v[t][:])
                    else:
                        nc.vector.affine_mul_reduce(
                            out=e1[:],
                            accum_out=res_sb[:, t * NRES + 3 : t * NRES + 4],
                            in0=e2v[t][:],
                            in1=e2v[t][:],
                            scale=1.0,
                            bias=0.0,
                        )

                def emit_e0(t):
                    e0 = scr.tile([128, NIB * TS], f16, tag="e0")
                    e0v[t] = e0
                    if not PROBE["no_vsq"]:
                        nc.vector.tensor_mul(e0[:], e1v[t][:], e1v[t][:])

                def emit_red(t, val, slot):
                    # Column-tiled (128x32 mode): chunk k runs in PE column
                    # group k, out -> PSUM partition 32k; the 4 chunks execute
                    # concurrently in the array.  lhsT = w_t * ones [128, 1].
                    if PROBE["no_red"]:
                        return
                    c = _wclass(t)
                    lhs = rw_sb[:, c : c + 1]
                    last = slot == 2 and t == NTILES - 1
                    for k in range(NIB):
                        nc.tensor.matmul(
                            acc_ps[32 * k : 32 * k + 1, :],
                            lhs,
                            val[:, k * TS : (k + 1) * TS],
                            start=state["first"][k],
                            stop=last,
                            skip_group_check=True,
                            tile_position=(0, 32 * k),
                        )
                        state["first"][k] = False

                for u in range(NTILES + 2):
                    if u < NTILES:
                        emit_mains(u)
                        emit_exp(u)
                        emit_e3(u)
                    if 1 <= u <= NTILES:
                        t = u - 1
                        emit_sq2(t)
                        emit_e1(t)
                        emit_e0(t)
                        emit_red(t, e3v.pop(t), 0)
                    if u >= 2:
                        t = u - 2
                        if E1_VIA_PE:
                            emit_red(t, e1v.pop(t), 1)
                        emit_red(t, e0v.pop(t), 2)

            if repeat == 1:
                body()
            else:
                with tc.For_i(0, repeat) as _i:
                    body()

            if PROBE["no_red"]:
                nc.vector.memset(acc_sb[:], 0.0)
            else:
                nc.vector.tensor_copy(acc_sb[:], acc_ps[:])
            nc.sync.dma_start(out=res[:], in_=res_sb[:])
            nc.sync.dma_start(out=accd[:], in_=acc_sb[:])

    nc.finalize()
    return nc


def _get_program():
    if "nc" not in _CACHE:
        _CACHE["nc"] = _build_program()
    return _CACHE["nc"]


def _core_tiles(k):
    """Per-core tile list: (rowbase, colbase, weight). Order defines t."""
    P = TS * k  # S row-block k
    Q = B + TS * (7 - k)  # T row-block 7-k
    tiles = [(P, P, 1.0), (Q, Q, 1.0)]  # SSd, TTd
    for j in range(k + 1, 8):  # SS+ (7-k tiles)
        tiles.append((P, TS * j, 2.0))
    for j in range(8 - k, 8):  # TT+ (k tiles)
        tiles.append((Q, B + TS * j, 2.0))
    for j in range(8):  # ST (8 tiles)
        tiles.append((P, B + TS * j, -2.0))
    assert len(tiles) == NTILES
    return tiles


def _host_prep(source_features, target_features):
    x = np.concatenate(
        [np.asarray(source_features, np.float32), np.asarray(target_features, np.float32)],
        axis=0,
    )  # [N, D]
    x64 = x.astype(np.float64)
    sq = np.sum(x64 * x64, axis=1)
    colsum = np.sum(x64, axis=0)
    sum_l2 = 2.0 * N * np.sum(sq) - 2.0 * np.dot(colsum, colsum)
    bandwidth = sum_l2 / (N * N - N) / (KERNEL_MUL ** (KERNEL_NUM // 2))
    a4 = 1.0 / (bandwidth * KERNEL_MUL**4)

    xt = np.ascontiguousarray(x.T)  # [D, N]
    sqf = sq.astype(np.float32)
    scale_host = np.full((128, 1), 2.0 * a4, np.float32)
    rw_host = np.zeros((128, 3), np.float16)
    for c, w in enumerate((1.0, 2.0, -2.0)):
        rw_host[:, c] = w
    AUGW = NUSLOT * 128 + NTILES * TS

    in_maps = []
    for k in range(NCORES):
        tiles = _core_tiles(k)
        rhs_host = np.empty((128, NTILES, 2, TS), np.float32)
        w_host = np.empty((128, NWB * NIB, 2, 128), np.float32)
        aug_host = np.empty((2, AUGW), np.float32)
        for t, (rb, cb, _w) in enumerate(tiles):
            rhs_host[:, t, 0, :] = xt[0:128, cb : cb + TS]
            rhs_host[:, t, 1, :] = xt[128:256, cb : cb + TS]
            v0 = NUSLOT * 128 + t * TS
            aug_host[0, v0 : v0 + TS] = -0.5 * sqf[cb : cb + TS]
            aug_host[1, v0 : v0 + TS] = 1.0
            for ib in range(NIB):
                r0 = rb + ib * 128
                us = _uslot(t, ib)
                aug_host[0, us * 128 : (us + 1) * 128] = 1.0
                aug_host[1, us * 128 : (us + 1) * 128] = -0.5 * sqf[r0 : r0 + 128]
                if 2 <= t <= 8:
                    w_host[:, (t - 2) * NIB + ib, 0, :] = xt[0:128, r0 : r0 + 128]
                    w_host[:, (t - 2) * NIB + ib, 1, :] = xt[128:256, r0 : r0 + 128]
        in_maps.append(
            {
                "xT": rhs_host,
                "wT": w_host,
                "aug2": aug_host,
                "scale": scale_host,
                "rw": rw_host,
            }
        )
    return in_maps


def _combine(results):
    total = 0.0
    for k in range(NCORES):
        r = np.asarray(results[k]["res"], np.float64).reshape(128, NTILES, NRES)
        # slots: 0,1 = b4 halves; 2 = b2; 3 = b1 (amr mode only)
        nslots = 3 if E1_VIA_PE else 4
        s_t = r[:, :, :nslots].sum(axis=(0, 2))
        w = np.array([w for (_rb, _cb, w) in _core_tiles(k)])
        total += float(np.dot(w, s_t))
        # accd: column-tiled reduce sums live in rows {0, 32, 64, 96}
        a = np.asarray(results[k]["accd"], np.float64)  # [128, TS] w-weighted
        total += float(a[0::32].sum())
    return np.float32(total / (B * B))


def kernel(source_features, target_features):
    from concourse.bass_utils import run_bass_kernel_spmd

    nc = _get_program()
    in_maps = _host_prep(source_features, target_features)
    out = run_bass_kernel_spmd(nc, in_maps, list(range(NCORES)))
    return _combine(out.results)
